# revision 1
# baseline (speedup 1.0000x reference)
"""Trainium2 Bass kernel for nn_GCBlock (gnn_message_passing).

Strategy: pure data-parallel over batch (2048 -> 8 cores x 256), with the
whole per-sample pipeline done in a transposed (time-on-partition) layout:

  h = LN_v( FC_t( AL[b] @ x[b] + gated banded temporal terms ) ) * alpha
      + beta + x[b]

- gate (gumbel straight-through) computed on CPU (tiny), folded into per-b
  joint-mixing matrix AL[b] = A1 + g2[b]*A3 and per-(b,v) gate patterns.
- per-b fused transpose matmuls: lhsT = x[b] half, rhs = [AL^T | I66]
  produce (AL@x)^T and x^T directly in PSUM (batched, 8 samples/group).
- banded temporal ops (adj_t, adj_tj) via constant shift matmuls (M2^T,
  S_up, S_dn + seam matrices) and batched vector ops.
- temporal FC via PSUM-accumulated matmuls streaming 3 rhs tensors.
- LN over joints = free-dim segmented reduces in transposed layout; affine
  per-sample normalize on ScalarE with per-partition scale/bias.
- output transposed back to natural layout on TensorE, DMA'd from PSUM.
"""
import numpy as np

B, V, T, J = 2048, 66, 256, 22
N_CORES = 8
BL = B // N_CORES          # 256 samples per core
NB = 8                     # samples per group
NG = BL // NB              # 32 groups
FD = NB * V                # 528 batched free dim
HC = FD // 2               # 264 per col-half

_NC_CACHE = {}


def _build_nc():
    if "nc" in _NC_CACHE:
        return _NC_CACHE["nc"]
    import concourse.bacc as bacc
    import concourse.mybir as mybir
    import concourse.tile as tile

    f32 = mybir.dt.float32
    Alu = mybir.AluOpType
    Act = mybir.ActivationFunctionType

    nc = bacc.Bacc("TRN2", target_bir_lowering=False, debug=False,
                   num_devices=N_CORES)

    xs = nc.dram_tensor("xs", [BL, V, T], f32, kind="ExternalInput").ap()
    alt = nc.dram_tensor("alt", [BL, V, V], f32, kind="ExternalInput").ap()
    gpat = nc.dram_tensor("gpat", [NG, 2, FD], f32, kind="ExternalInput").ap()
    m2t = nc.dram_tensor("m2t", [2, 128, 128], f32, kind="ExternalInput").ap()
    zm = nc.dram_tensor("zm", [2, 128, 128], f32, kind="ExternalInput").ap()
    sud = nc.dram_tensor("sud", [2, 128, 128], f32, kind="ExternalInput").ap()
    zs = nc.dram_tensor("zs", [2, 128, 128], f32, kind="ExternalInput").ap()
    i66 = nc.dram_tensor("i66", [V, V], f32, kind="ExternalInput").ap()
    i128 = nc.dram_tensor("i128", [128, 128], f32, kind="ExternalInput").ap()
    wq = nc.dram_tensor("wq", [2, 2, 128, 128], f32, kind="ExternalInput").ap()
    at3 = nc.dram_tensor("at3", [2, 2, 128, FD], f32, kind="ExternalInput").ap()
    arep = nc.dram_tensor("arep", [128, FD], f32, kind="ExternalInput").ap()
    brep = nc.dram_tensor("brep", [128, FD], f32, kind="ExternalInput").ap()
    fcb = nc.dram_tensor("fcb", [2, 128, 1], f32, kind="ExternalInput").ap()
    ys = nc.dram_tensor("ys", [BL, V, T], f32, kind="ExternalOutput").ap()

    with tile.TileContext(nc) as tc:
        import contextlib
        with contextlib.ExitStack() as ctx:
            cpool = ctx.enter_context(tc.tile_pool(name="consts", bufs=1))
            xpool = ctx.enter_context(tc.tile_pool(name="xin", bufs=6))
            apool = ctx.enter_context(tc.tile_pool(name="altin", bufs=6))
            gpool = ctx.enter_context(tc.tile_pool(name="greps", bufs=2))
            spool = ctx.enter_context(tc.tile_pool(name="sbwork", bufs=2))
            stpool = ctx.enter_context(tc.tile_pool(name="stats", bufs=2))
            pp = ctx.enter_context(tc.tile_pool(name="ps", bufs=1, space="PSUM"))

            # ---- constants ----
            c_m2t = [cpool.tile([128, 128], f32, name=f"cm2t{k}", tag=f"cm2t{k}") for k in range(2)]
            c_zm = [cpool.tile([128, 128], f32, name=f"czm{k}", tag=f"czm{k}") for k in range(2)]
            c_sud = [cpool.tile([128, 128], f32, name=f"csud{k}", tag=f"csud{k}") for k in range(2)]
            c_zs = [cpool.tile([128, 128], f32, name=f"czs{k}", tag=f"czs{k}") for k in range(2)]
            c_i128 = cpool.tile([128, 128], f32, name="ci128", tag="ci128")
            for h in range(2):
                nc.sync.dma_start(c_m2t[h][:], m2t[h])
                nc.sync.dma_start(c_zm[h][:], zm[h])
                nc.sync.dma_start(c_sud[h][:], sud[h])
                nc.sync.dma_start(c_zs[h][:], zs[h])
            nc.sync.dma_start(c_i128[:], i128[:])
            c_i66 = cpool.tile([V, V], f32, name="ci66", tag="ci66")
            nc.sync.dma_start(c_i66[:], i66[:])
            c_wq = [[cpool.tile([128, 128], f32, name=f"cwq{kh}{F}", tag=f"cwq{kh}{F}")
                     for F in range(2)] for kh in range(2)]
            for kh in range(2):
                for F in range(2):
                    nc.sync.dma_start(c_wq[kh][F][:], wq[kh, F])
            c_at3 = [[cpool.tile([128, FD], f32, name=f"cat3{d}{h}", tag=f"cat3{d}{h}")
                      for h in range(2)] for d in range(2)]
            for d in range(2):
                for h in range(2):
                    nc.sync.dma_start(c_at3[d][h][:], at3[d, h])
            c_arep = cpool.tile([128, FD], f32, name="carep", tag="carep")
            nc.sync.dma_start(c_arep[:], arep[:])
            c_brep = cpool.tile([128, FD], f32, name="cbrep", tag="cbrep")
            nc.sync.dma_start(c_brep[:], brep[:])
            c_fcb = [cpool.tile([128, 1], f32, name=f"cfcb{F}", tag=f"cfcb{F}") for F in range(2)]
            for F in range(2):
                nc.sync.dma_start(c_fcb[F][:], fcb[F])
            c_eps = cpool.tile([128, 1], f32, name="teps", tag="teps")
            nc.gpsimd.memset(c_eps[:], 1e-5)

            for g in range(NG):
                # ---- gate pattern replication ----
                grow = gpool.tile([1, 2 * FD], f32, name="t12", tag="grow")
                nc.sync.dma_start(grow[:], gpat[g].rearrange("a b -> (a b)").unsqueeze(0))
                g1r = gpool.tile([128, FD], f32, name="t13", tag="g1r")
                g3r = gpool.tile([128, FD], f32, name="t14", tag="g3r")
                nc.gpsimd.partition_broadcast(g1r[:], grow[:, 0:FD])
                nc.gpsimd.partition_broadcast(g3r[:], grow[:, FD:2 * FD])

                # ---- stage A: load + fused transpose matmuls ----
                pXM = [[pp.tile([128, HC], f32, name="t15", tag="pxm", bufs=2)
                        for _ in range(2)] for _ in range(2)]
                pXT = [[pp.tile([128, HC], f32, name="t16", tag="pxt", bufs=2)
                        for _ in range(2)] for _ in range(2)]
                for i in range(NB):
                    b = g * NB + i
                    xn = xpool.tile([V, T], f32, name="t17", tag="xn")
                    nc.sync.dma_start(xn[:], xs[b])
                    ab = apool.tile([V, V], f32, name="t18", tag="ab")
                    nc.sync.dma_start(ab[:], alt[b])
                    c, j = i // 4, i % 4
                    for h in range(2):
                        lhs = xn[:, 128 * h:128 * (h + 1)]
                        nc.tensor.matmul(pXM[h][c][:, 66 * j:66 * (j + 1)],
                                         lhs, ab[:], start=True, stop=True)
                        nc.tensor.matmul(pXT[h][c][:, 66 * j:66 * (j + 1)],
                                         lhs, c_i66[:], start=True, stop=True)

                # ---- stage B: copy XT to SBUF (batched) ----
                sXT = [spool.tile([128, FD], f32, name="t19", tag="sxt") for _ in range(2)]
                sXM = [spool.tile([128, FD], f32, name="t20", tag="sxm") for _ in range(2)]
                for h in range(2):
                    for c in range(2):
                        nc.scalar.copy(sXT[h][:, HC * c:HC * (c + 1)], pXT[h][c][:])
                        nc.scalar.copy(sXM[h][:, HC * c:HC * (c + 1)], pXM[h][c][:])

                # ---- stage C: banded shift matmuls ----
                pB = [[pp.tile([128, HC], f32, name="t21", tag="pband", bufs=2)
                       for _ in range(2)] for _ in range(2)]
                pSL = [[pp.tile([128, HC], f32, name="t22", tag="pband", bufs=2)
                        for _ in range(2)] for _ in range(2)]
                pSR = [[pp.tile([128, HC], f32, name="t23", tag="pband", bufs=2)
                        for _ in range(2)] for _ in range(2)]
                for h in range(2):
                    for c in range(2):
                        rhs_own = sXT[h][:, HC * c:HC * (c + 1)]
                        rhs_oth = sXT[1 - h][:, HC * c:HC * (c + 1)]
                        nc.tensor.matmul(pB[h][c][:], c_m2t[h][:], rhs_own,
                                         start=True, stop=False)
                        nc.tensor.matmul(pB[h][c][:], c_zm[h][:], rhs_oth,
                                         start=False, stop=True)
                        nc.tensor.matmul(pSL[h][c][:], c_sud[0][:], rhs_own,
                                         start=True, stop=(h == 0))
                        if h == 1:
                            nc.tensor.matmul(pSL[h][c][:], c_zs[0][:], rhs_oth,
                                             start=False, stop=True)
                        nc.tensor.matmul(pSR[h][c][:], c_sud[1][:], rhs_own,
                                         start=True, stop=(h == 1))
                        if h == 0:
                            nc.tensor.matmul(pSR[h][c][:], c_zs[1][:], rhs_oth,
                                             start=False, stop=True)

                # ---- stage D: banded vector ops ----
                band2 = [spool.tile([128, FD], f32, name="t24", tag="band2") for _ in range(2)]
                x4g = [spool.tile([128, FD], f32, name="t25", tag="x4g") for _ in range(2)]
                w3 = [spool.tile([128, FD], f32, name="t26", tag="w3") for _ in range(2)]
                w4 = [spool.tile([128, FD], f32, name="t27", tag="w4") for _ in range(2)]
                for h in range(2):
                    for c in range(2):
                        sl_ = slice(HC * c, HC * (c + 1))
                        nc.vector.tensor_tensor(band2[h][:, sl_], pB[h][c][:],
                                                g1r[:, sl_], Alu.mult)
                        nc.vector.tensor_tensor(w3[h][:, sl_], pSL[h][c][:],
                                                c_at3[0][h][:, sl_], Alu.mult)
                        nc.vector.tensor_tensor(w4[h][:, sl_], pSR[h][c][:],
                                                c_at3[1][h][:, sl_], Alu.mult)
                    nc.gpsimd.tensor_tensor(x4g[h][:], w3[h][:], w4[h][:], Alu.add)
                    nc.gpsimd.tensor_tensor(x4g[h][:], x4g[h][:], g3r[:], Alu.mult)

                # ---- stage E: FC with psum accumulation over kh and streams --
                pH = [[pp.tile([128, HC], f32, name="t28", tag="phh", bufs=2)
                       for _ in range(2)] for _ in range(2)]
                for F in range(2):
                    for c in range(2):
                        sl_ = slice(HC * c, HC * (c + 1))
                        first = True
                        for kh in range(2):
                            for stream in (sXM, band2, x4g):
                                nc.tensor.matmul(
                                    pH[F][c][:], c_wq[kh][F][:],
                                    stream[kh][:, sl_],
                                    start=first,
                                    stop=(kh == 1 and stream is x4g))
                                first = False

                # ---- stage F: LN tail ----
                ssq = [spool.tile([128, FD], f32, name="t29", tag="ssq") for _ in range(2)]
                mr = [stpool.tile([128, NB], f32, name="t30", tag="mr") for _ in range(2)]
                qr = [stpool.tile([128, NB], f32, name="t31", tag="qr") for _ in range(2)]
                for F in range(2):
                    for c in range(2):
                        sl_ = slice(HC * c, HC * (c + 1))
                        nc.scalar.square(ssq[F][:, sl_], pH[F][c][:])
                        nc.vector.tensor_reduce(
                            mr[F][:, 4 * c:4 * (c + 1)],
                            pH[F][c][:].rearrange("p (n v) -> p n v", n=4),
                            mybir.AxisListType.X, Alu.add)
                        nc.vector.tensor_reduce(
                            qr[F][:, 4 * c:4 * (c + 1)],
                            ssq[F][:, sl_].rearrange("p (n v) -> p n v", n=4),
                            mybir.AxisListType.X, Alu.add)
                mean = [stpool.tile([128, NB], f32, name="t32", tag="mean") for _ in range(2)]
                rstd = [stpool.tile([128, NB], f32, name="t33", tag="rstd") for _ in range(2)]
                negmr = [stpool.tile([128, NB], f32, name="t34", tag="negmr") for _ in range(2)]
                tmp = [stpool.tile([128, NB], f32, name="t35", tag="tmp") for _ in range(2)]
                for F in range(2):
                    nc.vector.tensor_scalar_mul(mean[F][:], mr[F][:], 1.0 / V)
                    nc.vector.tensor_scalar_mul(qr[F][:], qr[F][:], 1.0 / V)
                    nc.vector.tensor_tensor(tmp[F][:], mean[F][:], mean[F][:],
                                            Alu.mult)
                    nc.vector.tensor_tensor(tmp[F][:], qr[F][:], tmp[F][:],
                                            Alu.subtract)
                    nc.scalar.activation(tmp[F][:], tmp[F][:],
                                         Act.Sqrt, bias=c_eps[:])
                    nc.vector.reciprocal(rstd[F][:], tmp[F][:])
                    # negmr = (fcb - mean) * rstd
                    nc.vector.scalar_tensor_tensor(
                        negmr[F][:], mean[F][:], -1.0,
                        c_fcb[F][:].broadcast_to([128, NB]),
                        Alu.mult, Alu.add)
                    nc.vector.tensor_tensor(negmr[F][:], negmr[F][:], rstd[F][:],
                                            Alu.mult)

                nv = [spool.tile([128, FD], f32, name="t36", tag="nv") for _ in range(2)]
                outt = [spool.tile([128, FD], f32, name="t37", tag="outt") for _ in range(2)]
                for F in range(2):
                    for c in range(2):
                        for jj in range(4):
                            i = 4 * c + jj
                            nc.scalar.activation(
                                nv[F][:, 66 * i:66 * (i + 1)],
                                pH[F][c][:, 66 * jj:66 * (jj + 1)],
                                Act.Identity,
                                bias=negmr[F][:, i:i + 1],
                                scale=rstd[F][:, i:i + 1])
                    # w = nv * alpha_rep ; bx = xT + beta_rep ; out = w + bx
                    nc.vector.tensor_tensor(nv[F][:], nv[F][:], c_arep[:],
                                            Alu.mult)
                    nc.gpsimd.tensor_tensor(outt[F][:], sXT[F][:], c_brep[:],
                                            Alu.add)
                    nc.vector.tensor_tensor(outt[F][:], outt[F][:], nv[F][:],
                                            Alu.add)

                # ---- stage G: transpose back + store ----
                for i in range(NB):
                    b = g * NB + i
                    onat = spool.tile([V, T], f32, name="t38", tag="onat", bufs=6)
                    for F in range(2):
                        pO = pp.tile([V, 128], f32, name="t39", tag="pband",
                                     bufs=2)
                        nc.tensor.matmul(pO[:],
                                         outt[F][:, 66 * i:66 * (i + 1)],
                                         c_i128[:], start=True, stop=True)
                        nc.vector.tensor_copy(onat[:, 128 * F:128 * (F + 1)],
                                              pO[:])
                    nc.sync.dma_start(ys[b], onat[:])

    nc.compile()
    _NC_CACHE["nc"] = nc
    return nc


def _gate_np(x, mlp, if_make_dynamic, tau):
    """Replicate the reference gating exactly (jax fp32 on CPU)."""
    import jax
    import jax.numpy as jnp

    if True:
        xj = jnp.asarray(x)
        prob = xj.mean(axis=1) @ jnp.asarray(mlp)
        if if_make_dynamic:
            u = jax.random.uniform(jax.random.key(42), prob.shape,
                                   minval=1e-10, maxval=1.0)
            gumbel = -jnp.log(-jnp.log(u))
            soft = jax.nn.softmax((prob + gumbel) / tau, axis=-1)
            hard = jax.nn.one_hot(jnp.argmax(soft, axis=-1), prob.shape[-1],
                                  dtype=soft.dtype)
            gate = hard + soft - soft
        else:
            gate = jnp.zeros_like(prob).at[:, 0].set(1.0)
        return np.asarray(gate, dtype=np.float32)


def kernel(x, mlp, adj_j, adj_t, adj_jc, adj_tj, fc_w, fc_b, alpha, beta,
           if_make_dynamic, tau):
    from concourse.bass_utils import run_bass_kernel_spmd

    x = np.asarray(x, dtype=np.float32)
    mlp = np.asarray(mlp, dtype=np.float32)
    adj_j = np.asarray(adj_j, dtype=np.float32)
    adj_t = np.asarray(adj_t, dtype=np.float32)
    adj_jc = np.asarray(adj_jc, dtype=np.float32)
    adj_tj = np.asarray(adj_tj, dtype=np.float32)
    fc_w = np.asarray(fc_w, dtype=np.float32)
    fc_b = np.asarray(fc_b, dtype=np.float32)
    alpha_v = np.asarray(alpha, dtype=np.float32).reshape(V)
    beta_v = np.asarray(beta, dtype=np.float32).reshape(V)

    gate = _gate_np(x, mlp, if_make_dynamic, tau)
    g1, g2, g3 = gate[:, 1], gate[:, 2], gate[:, 3]

    # joint mixing matrices
    A1 = np.kron(adj_j, np.eye(3, dtype=np.float32))          # [66, 66]
    A3 = np.zeros((V, V), dtype=np.float32)                   # block diag
    for j in range(J):
        A3[3 * j:3 * j + 3, 3 * j:3 * j + 3] = adj_jc[j]
    AL = A1[None] + g2[:, None, None] * A3[None]              # [B, 66, 66]
    alt_all = np.ascontiguousarray(AL.transpose(0, 2, 1))

    # banded temporal matrices
    idx = np.arange(T)
    band = (np.abs(idx[:, None] - idx[None, :]) == 1).astype(np.float32)
    M2 = adj_t * band
    m2t = np.stack([M2[h * 128:(h + 1) * 128, h * 128:(h + 1) * 128].T.copy()
                    for h in range(2)])
    zm = np.zeros((2, 128, 128), dtype=np.float32)
    zm[0][0, 127] = M2[127, 128]      # into h0 row127 from sXT[1] row0
    zm[1][127, 0] = M2[128, 127]      # into h1 row0 from sXT[0] row127
    sud = np.stack([np.eye(128, k=1, dtype=np.float32),
                    np.eye(128, k=-1, dtype=np.float32)])
    zs = np.zeros((2, 128, 128), dtype=np.float32)
    zs[0][127, 0] = 1.0               # shL h1 row0 = xT[127] (h0)
    zs[1][0, 127] = 1.0               # shR h0 row127 = xT[128] (h1)

    # per-node banded coefficients, transposed + group-replicated
    atj_lo = np.zeros((V, T), dtype=np.float32)
    atj_hi = np.zeros((V, T), dtype=np.float32)
    atj_lo[:, 1:] = adj_tj[:, np.arange(1, T), np.arange(0, T - 1)]
    atj_hi[:, :-1] = adj_tj[:, np.arange(0, T - 1), np.arange(1, T)]
    at3 = np.zeros((2, 2, 128, FD), dtype=np.float32)
    for h in range(2):
        blk_lo = atj_lo[:, h * 128:(h + 1) * 128].T   # [128, 66]
        blk_hi = atj_hi[:, h * 128:(h + 1) * 128].T
        at3[0, h] = np.tile(blk_lo, (1, NB))
        at3[1, h] = np.tile(blk_hi, (1, NB))

    wqq = np.zeros((2, 2, 128, 128), dtype=np.float32)
    for kh in range(2):
        for F in range(2):
            wqq[kh, F] = fc_w[128 * F:128 * (F + 1),
                              128 * kh:128 * (kh + 1)].T.copy()
    arep = np.tile(alpha_v[None, :], (128, NB)).astype(np.float32)
    brep = np.tile(beta_v[None, :], (128, NB)).astype(np.float32)
    fcb = np.stack([fc_b[0:128, None], fc_b[128:256, None]]).astype(np.float32)

    i66m = np.eye(V, dtype=np.float32)
    i128m = np.eye(128, dtype=np.float32)

    in_maps = []
    for cidx in range(N_CORES):
        sl_ = slice(cidx * BL, (cidx + 1) * BL)
        g1c, g3c = g1[sl_], g3[sl_]
        gpat_c = np.zeros((NG, 2, FD), dtype=np.float32)
        gpat_c[:, 0, :] = np.repeat(g1c.reshape(NG, NB), V, axis=1)
        gpat_c[:, 1, :] = np.repeat(g3c.reshape(NG, NB), V, axis=1)
        in_maps.append(dict(
            xs=np.ascontiguousarray(x[sl_]),
            alt=np.ascontiguousarray(alt_all[sl_]),
            gpat=gpat_c, m2t=m2t, zm=zm, sud=sud, zs=zs,
            i66=i66m, i128=i128m, wq=wqq, at3=at3,
            arep=arep, brep=brep, fcb=fcb,
        ))

    nc = _build_nc()
    res = run_bass_kernel_spmd(nc, in_maps, core_ids=list(range(N_CORES)),
                               **_RUN_KW)
    _LAST_RES.clear()
    _LAST_RES["res"] = res
    out = np.empty((B, V, T), dtype=np.float32)
    for cidx in range(N_CORES):
        out[cidx * BL:(cidx + 1) * BL] = res.results[cidx]["ys"]
    return out


_RUN_KW = {}
_LAST_RES = {}



# revision 2
# speedup vs baseline: 1.3163x; 1.3163x over previous
"""Trainium2 Bass kernel for nn_GCBlock (gnn_message_passing).

Strategy: pure data-parallel over batch (2048 -> 8 cores x 256), with the
whole per-sample pipeline done in a transposed (time-on-partition) layout:

  h = LN_v( FC_t( AL[b] @ x[b] + gated banded temporal terms ) ) * alpha
      + beta + x[b]

- gate (gumbel straight-through) computed on CPU (tiny), folded into per-b
  joint-mixing matrix AL[b] = A1 + g2[b]*A3 and per-(b,v) gate patterns.
- all matmuls in bf16 (1 cycle/row on PE vs 4 for fp32), fp32 PSUM accum.
- per-b fused transpose matmuls: lhsT = x[b] half, rhs = [AL^T | I66]
  produce (AL@x)^T and x^T directly in PSUM (batched, 8 samples/group).
- banded temporal ops (adj_t, adj_tj) via constant shift matmuls (M2^T,
  S_up, S_dn + seam matrices) and batched vector ops.
- temporal FC via PSUM-accumulated matmuls streaming 3 rhs tensors.
- LN over joints = free-dim segmented reduces in transposed layout; affine
  per-sample normalize on ScalarE with per-partition scale/bias.
- output transposed back to natural layout on TensorE, DMA'd from PSUM.
"""
import numpy as np

B, V, T, J = 2048, 66, 256, 22
N_CORES = 8
BL = B // N_CORES          # 256 samples per core
NB = 8                     # samples per group
NG = BL // NB              # 32 groups
FD = NB * V                # 528 batched free dim
HC = FD // 2               # 264 per col-half

_NC_CACHE = {}


def _build_nc():
    if "nc" in _NC_CACHE:
        return _NC_CACHE["nc"]
    import concourse.bacc as bacc
    import concourse.mybir as mybir
    import concourse.tile as tile

    f32 = mybir.dt.float32
    bf16 = mybir.dt.bfloat16
    Alu = mybir.AluOpType
    Act = mybir.ActivationFunctionType

    nc = bacc.Bacc("TRN2", target_bir_lowering=False, debug=False,
                   num_devices=N_CORES)

    xs = nc.dram_tensor("xs", [BL, V, T], bf16, kind="ExternalInput").ap()
    alt = nc.dram_tensor("alt", [BL, V, V], bf16, kind="ExternalInput").ap()
    gpat = nc.dram_tensor("gpat", [NG, 2, FD], f32, kind="ExternalInput").ap()
    m2t = nc.dram_tensor("m2t", [2, 128, 128], bf16, kind="ExternalInput").ap()
    zm = nc.dram_tensor("zm", [2, 128, 128], bf16, kind="ExternalInput").ap()
    sud = nc.dram_tensor("sud", [2, 128, 128], bf16, kind="ExternalInput").ap()
    zs = nc.dram_tensor("zs", [2, 128, 128], bf16, kind="ExternalInput").ap()
    i66 = nc.dram_tensor("i66", [V, V], bf16, kind="ExternalInput").ap()
    i128 = nc.dram_tensor("i128", [128, 128], bf16, kind="ExternalInput").ap()
    wq = nc.dram_tensor("wq", [2, 2, 128, 128], bf16, kind="ExternalInput").ap()
    at3 = nc.dram_tensor("at3", [2, 2, 128, FD], f32, kind="ExternalInput").ap()
    arep = nc.dram_tensor("arep", [128, FD], f32, kind="ExternalInput").ap()
    brep = nc.dram_tensor("brep", [128, FD], bf16, kind="ExternalInput").ap()
    fcb = nc.dram_tensor("fcb", [2, 128, 1], f32, kind="ExternalInput").ap()
    ys = nc.dram_tensor("ys", [BL, V, T], f32, kind="ExternalOutput").ap()

    with tile.TileContext(nc) as tc:
        import contextlib
        with contextlib.ExitStack() as ctx:
            cpool = ctx.enter_context(tc.tile_pool(name="consts", bufs=1))
            xpool = ctx.enter_context(tc.tile_pool(name="xin", bufs=6))
            apool = ctx.enter_context(tc.tile_pool(name="altin", bufs=6))
            gpool = ctx.enter_context(tc.tile_pool(name="greps", bufs=2))
            spool = ctx.enter_context(tc.tile_pool(name="sbwork", bufs=2))
            stpool = ctx.enter_context(tc.tile_pool(name="stats", bufs=2))
            pp = ctx.enter_context(tc.tile_pool(name="ps", bufs=1, space="PSUM"))

            # ---- constants ----
            c_m2t = [cpool.tile([128, 128], bf16, name=f"cm2t{k}", tag=f"cm2t{k}") for k in range(2)]
            c_zm = [cpool.tile([128, 128], bf16, name=f"czm{k}", tag=f"czm{k}") for k in range(2)]
            c_sud = [cpool.tile([128, 128], bf16, name=f"csud{k}", tag=f"csud{k}") for k in range(2)]
            c_zs = [cpool.tile([128, 128], bf16, name=f"czs{k}", tag=f"czs{k}") for k in range(2)]
            c_i128 = cpool.tile([128, 128], bf16, name="ci128", tag="ci128")
            for h in range(2):
                nc.sync.dma_start(c_m2t[h][:], m2t[h])
                nc.sync.dma_start(c_zm[h][:], zm[h])
                nc.sync.dma_start(c_sud[h][:], sud[h])
                nc.sync.dma_start(c_zs[h][:], zs[h])
            nc.sync.dma_start(c_i128[:], i128[:])
            c_i66 = cpool.tile([V, V], bf16, name="ci66", tag="ci66")
            nc.sync.dma_start(c_i66[:], i66[:])
            c_wq = [[cpool.tile([128, 128], bf16, name=f"cwq{kh}{F}", tag=f"cwq{kh}{F}")
                     for F in range(2)] for kh in range(2)]
            for kh in range(2):
                for F in range(2):
                    nc.sync.dma_start(c_wq[kh][F][:], wq[kh, F])
            c_at3 = [[cpool.tile([128, FD], f32, name=f"cat3{d}{h}", tag=f"cat3{d}{h}")
                      for h in range(2)] for d in range(2)]
            for d in range(2):
                for h in range(2):
                    nc.sync.dma_start(c_at3[d][h][:], at3[d, h])
            c_arep = cpool.tile([128, FD], f32, name="carep", tag="carep")
            nc.sync.dma_start(c_arep[:], arep[:])
            c_brep = cpool.tile([128, FD], bf16, name="cbrep", tag="cbrep")
            nc.sync.dma_start(c_brep[:], brep[:])
            c_fcb = [cpool.tile([128, 1], f32, name=f"cfcb{F}", tag=f"cfcb{F}") for F in range(2)]
            for F in range(2):
                nc.sync.dma_start(c_fcb[F][:], fcb[F])
            c_eps = cpool.tile([128, 1], f32, name="teps", tag="teps")
            nc.gpsimd.memset(c_eps[:], 1e-5)

            for g in range(NG):
                # ---- gate pattern replication ----
                grow = gpool.tile([1, 2 * FD], f32, name="t12", tag="grow")
                nc.sync.dma_start(grow[:], gpat[g].rearrange("a b -> (a b)").unsqueeze(0))
                g1r = gpool.tile([128, FD], f32, name="t13", tag="g1r")
                g3r = gpool.tile([128, FD], f32, name="t14", tag="g3r")
                nc.gpsimd.partition_broadcast(g1r[:], grow[:, 0:FD])
                nc.gpsimd.partition_broadcast(g3r[:], grow[:, FD:2 * FD])
                g3rb = gpool.tile([128, FD], bf16, name="t42", tag="g3rb")
                nc.scalar.copy(g3rb[:], g3r[:])

                # ---- stage A: load + fused transpose matmuls ----
                pXM = [[pp.tile([128, HC], f32, name="t15", tag="pxm", bufs=2)
                        for _ in range(2)] for _ in range(2)]
                pXT = [[pp.tile([128, HC], f32, name="t16", tag="pxt", bufs=2)
                        for _ in range(2)] for _ in range(2)]
                for i in range(NB):
                    b = g * NB + i
                    xn = xpool.tile([V, T], bf16, name="t17", tag="xn")
                    nc.sync.dma_start(xn[:], xs[b])
                    ab = apool.tile([V, V], bf16, name="t18", tag="ab")
                    nc.sync.dma_start(ab[:], alt[b])
                    c, j = i // 4, i % 4
                    for h in range(2):
                        lhs = xn[:, 128 * h:128 * (h + 1)]
                        nc.tensor.matmul(pXM[h][c][:, 66 * j:66 * (j + 1)],
                                         lhs, ab[:], start=True, stop=True)
                        nc.tensor.matmul(pXT[h][c][:, 66 * j:66 * (j + 1)],
                                         lhs, c_i66[:], start=True, stop=True)

                # ---- stage B: copy XT to SBUF (batched) ----
                sXT = [spool.tile([128, FD], bf16, name="t19", tag="sxt") for _ in range(2)]
                sXM = [spool.tile([128, FD], bf16, name="t20", tag="sxm") for _ in range(2)]
                for h in range(2):
                    for c in range(2):
                        nc.scalar.copy(sXT[h][:, HC * c:HC * (c + 1)], pXT[h][c][:])
                        nc.scalar.copy(sXM[h][:, HC * c:HC * (c + 1)], pXM[h][c][:])

                # ---- stage C: banded shift matmuls ----
                pB = [[pp.tile([128, HC], f32, name="t21", tag="pband", bufs=2)
                       for _ in range(2)] for _ in range(2)]
                pSL = [[pp.tile([128, HC], f32, name="t22", tag="pband", bufs=2)
                        for _ in range(2)] for _ in range(2)]
                pSR = [[pp.tile([128, HC], f32, name="t23", tag="pband", bufs=2)
                        for _ in range(2)] for _ in range(2)]
                for h in range(2):
                    for c in range(2):
                        rhs_own = sXT[h][:, HC * c:HC * (c + 1)]
                        rhs_oth = sXT[1 - h][:, HC * c:HC * (c + 1)]
                        nc.tensor.matmul(pB[h][c][:], c_m2t[h][:], rhs_own,
                                         start=True, stop=False)
                        nc.tensor.matmul(pB[h][c][:], c_zm[h][:], rhs_oth,
                                         start=False, stop=True)
                        nc.tensor.matmul(pSL[h][c][:], c_sud[0][:], rhs_own,
                                         start=True, stop=(h == 0))
                        if h == 1:
                            nc.tensor.matmul(pSL[h][c][:], c_zs[0][:], rhs_oth,
                                             start=False, stop=True)
                        nc.tensor.matmul(pSR[h][c][:], c_sud[1][:], rhs_own,
                                         start=True, stop=(h == 1))
                        if h == 0:
                            nc.tensor.matmul(pSR[h][c][:], c_zs[1][:], rhs_oth,
                                             start=False, stop=True)

                # ---- stage D: banded vector ops ----
                band2 = [spool.tile([128, FD], bf16, name="t24", tag="band2") for _ in range(2)]
                x4g = [spool.tile([128, FD], bf16, name="t25", tag="x4g") for _ in range(2)]
                w3 = [spool.tile([128, FD], bf16, name="t26", tag="w3") for _ in range(2)]
                w4 = [spool.tile([128, FD], bf16, name="t27", tag="w4") for _ in range(2)]
                for h in range(2):
                    for c in range(2):
                        sl_ = slice(HC * c, HC * (c + 1))
                        nc.vector.tensor_tensor(band2[h][:, sl_], pB[h][c][:],
                                                g1r[:, sl_], Alu.mult)
                        nc.vector.tensor_tensor(w3[h][:, sl_], pSL[h][c][:],
                                                c_at3[0][h][:, sl_], Alu.mult)
                        nc.vector.tensor_tensor(w4[h][:, sl_], pSR[h][c][:],
                                                c_at3[1][h][:, sl_], Alu.mult)
                    nc.gpsimd.tensor_tensor(x4g[h][:], w3[h][:], w4[h][:], Alu.add)
                    nc.gpsimd.tensor_tensor(x4g[h][:], x4g[h][:], g3rb[:], Alu.mult)

                # ---- stage E: FC with psum accumulation over kh and streams --
                pH = [[pp.tile([128, HC], f32, name="t28", tag="phh", bufs=2)
                       for _ in range(2)] for _ in range(2)]
                for F in range(2):
                    for c in range(2):
                        sl_ = slice(HC * c, HC * (c + 1))
                        first = True
                        for kh in range(2):
                            for stream in (sXM, band2, x4g):
                                nc.tensor.matmul(
                                    pH[F][c][:], c_wq[kh][F][:],
                                    stream[kh][:, sl_],
                                    start=first,
                                    stop=(kh == 1 and stream is x4g))
                                first = False

                # ---- stage F: LN tail ----
                ssq = [spool.tile([128, FD], f32, name="t29", tag="ssq") for _ in range(2)]
                mr = [stpool.tile([128, NB], f32, name="t30", tag="mr") for _ in range(2)]
                qr = [stpool.tile([128, NB], f32, name="t31", tag="qr") for _ in range(2)]
                for F in range(2):
                    for c in range(2):
                        sl_ = slice(HC * c, HC * (c + 1))
                        nc.scalar.square(ssq[F][:, sl_], pH[F][c][:])
                        nc.vector.tensor_reduce(
                            mr[F][:, 4 * c:4 * (c + 1)],
                            pH[F][c][:].rearrange("p (n v) -> p n v", n=4),
                            mybir.AxisListType.X, Alu.add)
                        nc.vector.tensor_reduce(
                            qr[F][:, 4 * c:4 * (c + 1)],
                            ssq[F][:, sl_].rearrange("p (n v) -> p n v", n=4),
                            mybir.AxisListType.X, Alu.add)
                mean = [stpool.tile([128, NB], f32, name="t32", tag="mean") for _ in range(2)]
                rstd = [stpool.tile([128, NB], f32, name="t33", tag="rstd") for _ in range(2)]
                negmr = [stpool.tile([128, NB], f32, name="t34", tag="negmr") for _ in range(2)]
                tmp = [stpool.tile([128, NB], f32, name="t35", tag="tmp") for _ in range(2)]
                for F in range(2):
                    nc.vector.tensor_scalar_mul(mean[F][:], mr[F][:], 1.0 / V)
                    nc.vector.tensor_scalar_mul(qr[F][:], qr[F][:], 1.0 / V)
                    nc.vector.tensor_tensor(tmp[F][:], mean[F][:], mean[F][:],
                                            Alu.mult)
                    nc.vector.tensor_tensor(tmp[F][:], qr[F][:], tmp[F][:],
                                            Alu.subtract)
                    nc.scalar.activation(tmp[F][:], tmp[F][:],
                                         Act.Sqrt, bias=c_eps[:])
                    nc.vector.reciprocal(rstd[F][:], tmp[F][:])
                    # negmr = (fcb - mean) * rstd
                    nc.vector.scalar_tensor_tensor(
                        negmr[F][:], mean[F][:], -1.0,
                        c_fcb[F][:].broadcast_to([128, NB]),
                        Alu.mult, Alu.add)
                    nc.vector.tensor_tensor(negmr[F][:], negmr[F][:], rstd[F][:],
                                            Alu.mult)

                nv = [spool.tile([128, FD], f32, name="t36", tag="nv") for _ in range(2)]
                nvb = [spool.tile([128, FD], bf16, name="t40", tag="nvb") for _ in range(2)]
                outt = [spool.tile([128, FD], bf16, name="t37", tag="outt") for _ in range(2)]
                for F in range(2):
                    for c in range(2):
                        for jj in range(4):
                            i = 4 * c + jj
                            nc.scalar.activation(
                                nv[F][:, 66 * i:66 * (i + 1)],
                                pH[F][c][:, 66 * jj:66 * (jj + 1)],
                                Act.Identity,
                                bias=negmr[F][:, i:i + 1],
                                scale=rstd[F][:, i:i + 1])
                    # w = nv * alpha_rep ; bx = xT + beta_rep ; out = w + bx
                    nc.vector.tensor_tensor(nvb[F][:], nv[F][:], c_arep[:],
                                            Alu.mult)
                    nc.gpsimd.tensor_tensor(outt[F][:], sXT[F][:], c_brep[:],
                                            Alu.add)
                    nc.vector.tensor_tensor(outt[F][:], outt[F][:], nvb[F][:],
                                            Alu.add)

                # ---- stage G: transpose back + store ----
                for i in range(NB):
                    b = g * NB + i
                    onat = spool.tile([V, T], f32, name="t38", tag="onat", bufs=6)
                    for F in range(2):
                        pO = pp.tile([V, 128], f32, name="t39", tag="pband",
                                     bufs=2)
                        nc.tensor.matmul(pO[:],
                                         outt[F][:, 66 * i:66 * (i + 1)],
                                         c_i128[:], start=True, stop=True)
                        nc.vector.tensor_copy(onat[:, 128 * F:128 * (F + 1)],
                                              pO[:])
                    nc.sync.dma_start(ys[b], onat[:])

    nc.compile()
    _NC_CACHE["nc"] = nc
    return nc


def _gate_np(x, mlp, if_make_dynamic, tau):
    """Replicate the reference gating exactly (jax fp32 on CPU)."""
    import jax
    import jax.numpy as jnp

    if True:
        xj = jnp.asarray(x)
        prob = xj.mean(axis=1) @ jnp.asarray(mlp)
        if if_make_dynamic:
            u = jax.random.uniform(jax.random.key(42), prob.shape,
                                   minval=1e-10, maxval=1.0)
            gumbel = -jnp.log(-jnp.log(u))
            soft = jax.nn.softmax((prob + gumbel) / tau, axis=-1)
            hard = jax.nn.one_hot(jnp.argmax(soft, axis=-1), prob.shape[-1],
                                  dtype=soft.dtype)
            gate = hard + soft - soft
        else:
            gate = jnp.zeros_like(prob).at[:, 0].set(1.0)
        return np.asarray(gate, dtype=np.float32)


def kernel(x, mlp, adj_j, adj_t, adj_jc, adj_tj, fc_w, fc_b, alpha, beta,
           if_make_dynamic, tau):
    from concourse.bass_utils import run_bass_kernel_spmd
    import ml_dtypes

    BF = ml_dtypes.bfloat16

    x = np.asarray(x, dtype=np.float32)
    mlp = np.asarray(mlp, dtype=np.float32)
    adj_j = np.asarray(adj_j, dtype=np.float32)
    adj_t = np.asarray(adj_t, dtype=np.float32)
    adj_jc = np.asarray(adj_jc, dtype=np.float32)
    adj_tj = np.asarray(adj_tj, dtype=np.float32)
    fc_w = np.asarray(fc_w, dtype=np.float32)
    fc_b = np.asarray(fc_b, dtype=np.float32)
    alpha_v = np.asarray(alpha, dtype=np.float32).reshape(V)
    beta_v = np.asarray(beta, dtype=np.float32).reshape(V)

    gate = _gate_np(x, mlp, if_make_dynamic, tau)
    g1, g2, g3 = gate[:, 1], gate[:, 2], gate[:, 3]

    # joint mixing matrices
    A1 = np.kron(adj_j, np.eye(3, dtype=np.float32))          # [66, 66]
    A3 = np.zeros((V, V), dtype=np.float32)                   # block diag
    for j in range(J):
        A3[3 * j:3 * j + 3, 3 * j:3 * j + 3] = adj_jc[j]
    AL = A1[None] + g2[:, None, None] * A3[None]              # [B, 66, 66]
    alt_all = np.ascontiguousarray(AL.transpose(0, 2, 1)).astype(BF)

    # banded temporal matrices
    idx = np.arange(T)
    band = (np.abs(idx[:, None] - idx[None, :]) == 1).astype(np.float32)
    M2 = adj_t * band
    m2t = np.stack([M2[h * 128:(h + 1) * 128, h * 128:(h + 1) * 128].T.copy()
                    for h in range(2)])
    zm = np.zeros((2, 128, 128), dtype=np.float32)
    zm[0][0, 127] = M2[127, 128]      # into h0 row127 from sXT[1] row0
    zm[1][127, 0] = M2[128, 127]      # into h1 row0 from sXT[0] row127
    sud = np.stack([np.eye(128, k=1, dtype=np.float32),
                    np.eye(128, k=-1, dtype=np.float32)])
    zs = np.zeros((2, 128, 128), dtype=np.float32)
    zs[0][127, 0] = 1.0               # shL h1 row0 = xT[127] (h0)
    zs[1][0, 127] = 1.0               # shR h0 row127 = xT[128] (h1)

    # per-node banded coefficients, transposed + group-replicated
    atj_lo = np.zeros((V, T), dtype=np.float32)
    atj_hi = np.zeros((V, T), dtype=np.float32)
    atj_lo[:, 1:] = adj_tj[:, np.arange(1, T), np.arange(0, T - 1)]
    atj_hi[:, :-1] = adj_tj[:, np.arange(0, T - 1), np.arange(1, T)]
    at3 = np.zeros((2, 2, 128, FD), dtype=np.float32)
    for h in range(2):
        blk_lo = atj_lo[:, h * 128:(h + 1) * 128].T   # [128, 66]
        blk_hi = atj_hi[:, h * 128:(h + 1) * 128].T
        at3[0, h] = np.tile(blk_lo, (1, NB))
        at3[1, h] = np.tile(blk_hi, (1, NB))

    wqq = np.zeros((2, 2, 128, 128), dtype=np.float32)
    for kh in range(2):
        for F in range(2):
            wqq[kh, F] = fc_w[128 * F:128 * (F + 1),
                              128 * kh:128 * (kh + 1)].T.copy()
    arep = np.tile(alpha_v[None, :], (128, NB)).astype(np.float32)
    brep = np.tile(beta_v[None, :], (128, NB)).astype(BF)
    fcb = np.stack([fc_b[0:128, None], fc_b[128:256, None]]).astype(np.float32)

    i66m = np.eye(V, dtype=np.float32)
    i128m = np.eye(128, dtype=np.float32)

    in_maps = []
    for cidx in range(N_CORES):
        sl_ = slice(cidx * BL, (cidx + 1) * BL)
        g1c, g3c = g1[sl_], g3[sl_]
        gpat_c = np.zeros((NG, 2, FD), dtype=np.float32)
        gpat_c[:, 0, :] = np.repeat(g1c.reshape(NG, NB), V, axis=1)
        gpat_c[:, 1, :] = np.repeat(g3c.reshape(NG, NB), V, axis=1)
        in_maps.append(dict(
            xs=np.ascontiguousarray(x[sl_]).astype(BF),
            alt=np.ascontiguousarray(alt_all[sl_]),
            gpat=gpat_c,
            m2t=m2t.astype(BF), zm=zm.astype(BF),
            sud=sud.astype(BF), zs=zs.astype(BF),
            i66=i66m.astype(BF), i128=i128m.astype(BF),
            wq=wqq.astype(BF), at3=at3,
            arep=arep, brep=brep, fcb=fcb,
        ))

    nc = _build_nc()
    res = run_bass_kernel_spmd(nc, in_maps, core_ids=list(range(N_CORES)),
                               **_RUN_KW)
    _LAST_RES.clear()
    _LAST_RES["res"] = res
    out = np.empty((B, V, T), dtype=np.float32)
    for cidx in range(N_CORES):
        out[cidx * BL:(cidx + 1) * BL] = res.results[cidx]["ys"]
    return out


_RUN_KW = {}
_LAST_RES = {}


# revision 8
# speedup vs baseline: 1.3765x; 1.0457x over previous
"""Trainium2 Bass kernel for nn_GCBlock (gnn_message_passing).

Data-parallel over batch (2048 -> 8 cores x 256). Per core, samples are
processed in 32 groups of 8, batched along the free dim in a transposed
(time-on-partition) layout. All matmuls bf16 (1 PE cycle/row), fp32 PSUM.

Work split per group:
- host ships x in BOTH layouts (natural for PE-stationary, pre-transposed
  for everything else), so no on-chip transpose of x is needed.
- joint mixing (A1 + g2*A3) folded on host into per-sample AL, applied as
  PE matmuls with natural-x stationary -> (AL@x)^T lands in PSUM.
- g1*x2 (banded adj_t) folded into a second FC matrix W2 = fc_w @ M2band;
  its stream input is just g1-gated x^T (one vector op).
- g3*x4 (per-node banded adj_tj) via partition-shifted SBUF->SBUF DMA
  copies of x^T plus 4 elementwise ops.
- FC = PSUM-accumulated matmuls over 3 streams x 2 k-halves.
- LN over joints: segmented reduces + stride-0-broadcast normalize.
- residual added in transposed layout; output shipped transposed and
  reassembled (transpose + f32 cast) on host.
"""
import numpy as np

B, V, T, J = 2048, 66, 256, 22
N_CORES = 8
BL = B // N_CORES          # 256 samples per core
NB = 8                     # samples per group
NG = BL // NB              # 32 groups
FD = NB * V                # 528 batched free dim
HC = FD // 2               # 264 per col-half

_NC_CACHE = {}


def _build_nc(trivial_affine):
    key = ("nc", trivial_affine)
    if key in _NC_CACHE:
        return _NC_CACHE[key]
    import concourse.bacc as bacc
    import concourse.mybir as mybir
    import concourse.tile as tile

    f32 = mybir.dt.float32
    bf16 = mybir.dt.bfloat16
    Alu = mybir.AluOpType
    Act = mybir.ActivationFunctionType

    nc = bacc.Bacc("TRN2", target_bir_lowering=False, debug=False,
                   num_devices=N_CORES)

    xsn = nc.dram_tensor("xsn", [NG, V, NB * T], bf16, kind="ExternalInput").ap()
    xst = nc.dram_tensor("xst", [NG, 2, 128, FD], bf16, kind="ExternalInput").ap()
    alt = nc.dram_tensor("alt", [NG, V, FD], bf16, kind="ExternalInput").ap()
    gpat = nc.dram_tensor("gpat", [NG, 2, FD], bf16, kind="ExternalInput").ap()
    lohi = nc.dram_tensor("lohi", [2, 2, 128, FD], bf16, kind="ExternalInput").ap()
    wqs = nc.dram_tensor("wqs", [2, 2, 2, 128, 128], bf16, kind="ExternalInput").ap()
    arep = nc.dram_tensor("arep", [128, FD], bf16, kind="ExternalInput").ap()
    brep = nc.dram_tensor("brep", [128, FD], bf16, kind="ExternalInput").ap()
    fcb = nc.dram_tensor("fcb", [2, 128, 1], f32, kind="ExternalInput").ap()
    yst = nc.dram_tensor("yst", [NG, 2, 128, FD], bf16, kind="ExternalOutput").ap()

    with tile.TileContext(nc) as tc:
        import contextlib
        with contextlib.ExitStack() as ctx:
            cpool = ctx.enter_context(tc.tile_pool(name="consts", bufs=1))
            xpool = ctx.enter_context(tc.tile_pool(name="xin", bufs=3))
            spool = ctx.enter_context(tc.tile_pool(name="sbwork", bufs=2))
            stpool = ctx.enter_context(tc.tile_pool(name="stats", bufs=2))
            pmx = ctx.enter_context(tc.tile_pool(name="psA", bufs=1, space="PSUM"))
            pph = ctx.enter_context(tc.tile_pool(name="psH", bufs=1, space="PSUM"))

            # ---- constants ----
            c_lo = [cpool.tile([128, FD], bf16, name=f"clo{h}", tag=f"clo{h}")
                    for h in range(2)]
            c_hi = [cpool.tile([128, FD], bf16, name=f"chi{h}", tag=f"chi{h}")
                    for h in range(2)]
            for h in range(2):
                nc.sync.dma_start(c_lo[h][:], lohi[0, h])
                nc.sync.dma_start(c_hi[h][:], lohi[1, h])
            c_wq = [[[cpool.tile([128, 128], bf16, name=f"cwq{w}{kh}{F}",
                                 tag=f"cwq{w}{kh}{F}")
                      for F in range(2)] for kh in range(2)] for w in range(2)]
            for w in range(2):
                for kh in range(2):
                    for F in range(2):
                        nc.sync.dma_start(c_wq[w][kh][F][:], wqs[w, kh, F])
            c_arep = cpool.tile([128, FD], bf16, name="carep", tag="carep")
            nc.sync.dma_start(c_arep[:], arep[:])
            c_brep = cpool.tile([128, FD], bf16, name="cbrep", tag="cbrep")
            nc.sync.dma_start(c_brep[:], brep[:])
            c_fcb = [cpool.tile([128, 1], f32, name=f"cfcb{F}", tag=f"cfcb{F}")
                     for F in range(2)]
            for F in range(2):
                nc.sync.dma_start(c_fcb[F][:], fcb[F])
            c_eps = cpool.tile([128, 1], f32, name="teps", tag="teps")
            nc.gpsimd.memset(c_eps[:], 1e-5)

            for g in range(NG):
                # ---- loads ----
                xn = xpool.tile([V, NB * T], bf16, name="t10", tag="xn")
                nc.sync.dma_start(xn[:], xsn[g])
                ab = xpool.tile([V, FD], bf16, name="t11", tag="ab")
                nc.sync.dma_start(ab[:], alt[g])
                sXT = [xpool.tile([128, FD], bf16, name="t12", tag=f"sxt{h}")
                       for h in range(2)]
                for h in range(2):
                    nc.sync.dma_start(sXT[h][:], xst[g, h])
                grow = xpool.tile([1, 2 * FD], bf16, name="t13", tag="grow")
                nc.sync.dma_start(grow[:],
                                  gpat[g].rearrange("a b -> (a b)").unsqueeze(0))
                g1rb = spool.tile([128, FD], bf16, name="t14", tag="g1rb")
                g3rb = spool.tile([128, FD], bf16, name="t15", tag="g3rb")
                nc.gpsimd.partition_broadcast(g1rb[:], grow[:, 0:FD])
                nc.gpsimd.partition_broadcast(g3rb[:], grow[:, FD:2 * FD])

                # ---- shifted copies of x^T (banded temporal taps) ----
                xlo = [spool.tile([128, FD], bf16, name="t16", tag=f"xlo{h}")
                       for h in range(2)]
                xhi = [spool.tile([128, FD], bf16, name="t17", tag=f"xhi{h}")
                       for h in range(2)]
                # xlo[h][f] = xT[128h+f-1]; row 0 of h=0 multiplies a zero coeff
                nc.sync.dma_start(xlo[0][1:128, :], sXT[0][0:127, :])
                nc.sync.dma_start(xlo[0][0:1, :], sXT[0][0:1, :])
                nc.sync.dma_start(xlo[1][1:128, :], sXT[1][0:127, :])
                nc.sync.dma_start(xlo[1][0:1, :], sXT[0][127:128, :])
                # xhi[h][f] = xT[128h+f+1]; row 127 of h=1 multiplies a zero coeff
                nc.sync.dma_start(xhi[0][0:127, :], sXT[0][1:128, :])
                nc.sync.dma_start(xhi[0][127:128, :], sXT[1][0:1, :])
                nc.sync.dma_start(xhi[1][0:127, :], sXT[1][1:128, :])
                nc.sync.dma_start(xhi[1][127:128, :], sXT[1][127:128, :])

                # ---- stage A: joint-mix matmuls (natural x stationary) ----
                pXM = [[pmx.tile([128, HC], f32, name="t18", tag=f"pxm{h}{c}")
                        for c in range(2)] for h in range(2)]
                for i in range(NB):
                    c, j = i // 4, i % 4
                    for h in range(2):
                        lhs = xn[:, T * i + 128 * h:T * i + 128 * (h + 1)]
                        nc.tensor.matmul(pXM[h][c][:, 66 * j:66 * (j + 1)],
                                         lhs, ab[:, 66 * i:66 * (i + 1)],
                                         start=True, stop=True)

                # ---- x4 stream: banded per-node taps ----
                w3 = [spool.tile([128, FD], bf16, name="t19", tag=f"w3{h}")
                      for h in range(2)]
                w4 = [spool.tile([128, FD], bf16, name="t20", tag=f"w4{h}")
                      for h in range(2)]
                x4s = [spool.tile([128, FD], bf16, name="t21", tag=f"x4s{h}")
                       for h in range(2)]
                x4g = [spool.tile([128, FD], bf16, name="t22", tag=f"x4g{h}")
                       for h in range(2)]
                for h in range(2):
                    nc.vector.tensor_tensor(w3[h][:], xlo[h][:], c_lo[h][:],
                                            Alu.mult)
                    nc.gpsimd.tensor_tensor(w4[h][:], xhi[h][:], c_hi[h][:],
                                            Alu.mult)
                    nc.vector.tensor_tensor(x4s[h][:], w3[h][:], w4[h][:],
                                            Alu.add)
                    nc.gpsimd.tensor_tensor(x4g[h][:], x4s[h][:], g3rb[:],
                                            Alu.mult)

                # ---- streams: s4 = (AL@x)^T + x4g (evac fused), gX1 ----
                s4 = [spool.tile([128, FD], bf16, name="t23", tag=f"s4{h}")
                      for h in range(2)]
                gX1 = [spool.tile([128, FD], bf16, name="t24", tag=f"gx1{h}")
                       for h in range(2)]
                for h in range(2):
                    for c in range(2):
                        sl_ = slice(HC * c, HC * (c + 1))
                        nc.vector.tensor_tensor(s4[h][:, sl_], pXM[h][c][:],
                                                x4g[h][:, sl_], Alu.add)
                    nc.gpsimd.tensor_tensor(gX1[h][:], sXT[h][:], g1rb[:],
                                            Alu.mult)

                # ---- stage E: FC via PSUM accumulation ----
                pH = [[pph.tile([128, HC], f32, name="t25", tag=f"phh{F}{c}")
                       for c in range(2)] for F in range(2)]
                for F in range(2):
                    for c in range(2):
                        sl_ = slice(HC * c, HC * (c + 1))
                        first = True
                        for kh in range(2):
                            nc.tensor.matmul(pH[F][c][:], c_wq[0][kh][F][:],
                                             s4[kh][:, sl_],
                                             start=first, stop=False)
                            first = False
                            nc.tensor.matmul(pH[F][c][:], c_wq[1][kh][F][:],
                                             gX1[kh][:, sl_],
                                             start=False, stop=(kh == 1))

                # ---- stage F: LN stats ----
                ssq = [spool.tile([128, FD], bf16, name="t26", tag=f"ssq{F}")
                       for F in range(2)]
                mr = [stpool.tile([128, NB], f32, name="t27", tag=f"mr{F}")
                      for F in range(2)]
                qr = [stpool.tile([128, NB], f32, name="t28", tag=f"qr{F}")
                      for F in range(2)]
                for F in range(2):
                    for c in range(2):
                        sl_ = slice(HC * c, HC * (c + 1))
                        nc.vector.tensor_reduce(
                            mr[F][:, 4 * c:4 * (c + 1)],
                            pH[F][c][:].rearrange("p (n v) -> p n v", n=4),
                            mybir.AxisListType.X, Alu.add)
                        # square with free-dim accumulate: per-sample E[h^2]
                        for jj in range(4):
                            i = 4 * c + jj
                            nc.scalar.activation(
                                ssq[F][:, 66 * i:66 * (i + 1)],
                                pH[F][c][:, 66 * jj:66 * (jj + 1)],
                                Act.Square,
                                accum_out=qr[F][:, i:i + 1])
                mean = [stpool.tile([128, NB], f32, name="t29", tag=f"mean{F}")
                        for F in range(2)]
                rstd = [stpool.tile([128, NB], f32, name="t30", tag=f"rstd{F}")
                        for F in range(2)]
                negm = [stpool.tile([128, NB], f32, name="t31", tag=f"negm{F}")
                        for F in range(2)]
                negmb = [stpool.tile([128, NB], bf16, name="t32", tag=f"negmb{F}")
                         for F in range(2)]
                tmp = [stpool.tile([128, NB], f32, name="t33", tag=f"tmp{F}")
                       for F in range(2)]
                for F in range(2):
                    nc.gpsimd.tensor_scalar_mul(mean[F][:], mr[F][:], 1.0 / V)
                    nc.gpsimd.tensor_tensor(tmp[F][:], mean[F][:], mean[F][:],
                                            Alu.mult)
                    # var = qr/V - mean^2 ; tmp := qr/V - mean^2 via stt
                    nc.vector.scalar_tensor_tensor(
                        tmp[F][:], qr[F][:], 1.0 / V, tmp[F][:],
                        Alu.mult, Alu.subtract)
                    nc.scalar.activation(tmp[F][:], tmp[F][:],
                                         Act.Sqrt, bias=c_eps[:])
                    nc.vector.reciprocal(rstd[F][:], tmp[F][:])
                    # negm = (fcb - mean) * rstd
                    nc.vector.scalar_tensor_tensor(
                        negm[F][:], mean[F][:], -1.0,
                        c_fcb[F][:].broadcast_to([128, NB]),
                        Alu.mult, Alu.add)
                    nc.gpsimd.tensor_tensor(negm[F][:], negm[F][:], rstd[F][:],
                                            Alu.mult)
                    nc.scalar.copy(negmb[F][:], negm[F][:])

                # ---- normalize + affine + residual (broadcast APs) ----
                tno = [spool.tile([128, FD], bf16, name="t34", tag=f"tno{F}")
                       for F in range(2)]
                u = [spool.tile([128, FD], bf16, name="t35", tag=f"u{F}")
                     for F in range(2)]
                outt = [spool.tile([128, FD], bf16, name="t36", tag=f"outt{F}")
                        for F in range(2)]
                for F in range(2):
                    for c in range(2):
                        sl_ = slice(HC * c, HC * (c + 1))
                        rbc = rstd[F][:, 4 * c:4 * (c + 1)] \
                            .unsqueeze(2).broadcast_to([128, 4, 66])
                        nc.vector.tensor_tensor(
                            tno[F][:, sl_].rearrange("p (n v) -> p n v", n=4),
                            pH[F][c][:].rearrange("p (n v) -> p n v", n=4),
                            rbc, Alu.mult)
                    nbc = negmb[F][:].unsqueeze(2).broadcast_to([128, NB, 66])
                    nc.gpsimd.tensor_tensor(
                        u[F][:].rearrange("p (n v) -> p n v", n=NB),
                        tno[F][:].rearrange("p (n v) -> p n v", n=NB),
                        nbc, Alu.add)
                    if trivial_affine:
                        nc.gpsimd.tensor_tensor(outt[F][:], u[F][:],
                                                sXT[F][:], Alu.add)
                    else:
                        nc.vector.tensor_tensor(u[F][:], u[F][:], c_arep[:],
                                                Alu.mult)
                        nc.gpsimd.tensor_tensor(outt[F][:], sXT[F][:],
                                                c_brep[:], Alu.add)
                        nc.gpsimd.tensor_tensor(outt[F][:], outt[F][:],
                                                u[F][:], Alu.add)
                    nc.sync.dma_start(yst[g, F], outt[F][:])

    nc.compile()
    _NC_CACHE[key] = nc
    return nc


def _gate_np(x, mlp, if_make_dynamic, tau):
    """Replicate the reference gating exactly (jax fp32 on CPU)."""
    import jax
    import jax.numpy as jnp

    xj = jnp.asarray(x)
    prob = xj.mean(axis=1) @ jnp.asarray(mlp)
    if if_make_dynamic:
        u = jax.random.uniform(jax.random.key(42), prob.shape,
                               minval=1e-10, maxval=1.0)
        gumbel = -jnp.log(-jnp.log(u))
        soft = jax.nn.softmax((prob + gumbel) / tau, axis=-1)
        hard = jax.nn.one_hot(jnp.argmax(soft, axis=-1), prob.shape[-1],
                              dtype=soft.dtype)
        gate = hard + soft - soft
    else:
        gate = jnp.zeros_like(prob).at[:, 0].set(1.0)
    return np.asarray(gate, dtype=np.float32)


def kernel(x, mlp, adj_j, adj_t, adj_jc, adj_tj, fc_w, fc_b, alpha, beta,
           if_make_dynamic, tau):
    from concourse.bass_utils import run_bass_kernel_spmd
    import ml_dtypes

    BF = ml_dtypes.bfloat16

    x = np.asarray(x, dtype=np.float32)
    mlp = np.asarray(mlp, dtype=np.float32)
    adj_j = np.asarray(adj_j, dtype=np.float32)
    adj_t = np.asarray(adj_t, dtype=np.float32)
    adj_jc = np.asarray(adj_jc, dtype=np.float32)
    adj_tj = np.asarray(adj_tj, dtype=np.float32)
    fc_w = np.asarray(fc_w, dtype=np.float32)
    fc_b = np.asarray(fc_b, dtype=np.float32)
    alpha_v = np.asarray(alpha, dtype=np.float32).reshape(V)
    beta_v = np.asarray(beta, dtype=np.float32).reshape(V)
    trivial_affine = bool(np.all(alpha_v == 1.0) and np.all(beta_v == 0.0))

    gate = _gate_np(x, mlp, if_make_dynamic, tau)
    g1, g2, g3 = gate[:, 1], gate[:, 2], gate[:, 3]

    # joint mixing matrices
    A1 = np.kron(adj_j, np.eye(3, dtype=np.float32))          # [66, 66]
    A3 = np.zeros((V, V), dtype=np.float32)                   # block diag
    for j in range(J):
        A3[3 * j:3 * j + 3, 3 * j:3 * j + 3] = adj_jc[j]
    AL = A1[None] + g2[:, None, None] * A3[None]              # [B, 66, 66]
    # per-group packed AL^T: alt[g][:, 66*i:66*(i+1)] = AL[b].T
    altT = np.ascontiguousarray(AL.transpose(0, 2, 1))        # [B, 66, 66]

    # banded temporal matrix folded into a second FC matrix
    idx = np.arange(T)
    band = (np.abs(idx[:, None] - idx[None, :]) == 1).astype(np.float32)
    M2 = adj_t * band
    W2 = fc_w @ M2                                            # [T, T]

    # per-node banded tap coefficients (transposed, group-replicated)
    atj_lo = np.zeros((V, T), dtype=np.float32)
    atj_hi = np.zeros((V, T), dtype=np.float32)
    atj_lo[:, 1:] = adj_tj[:, np.arange(1, T), np.arange(0, T - 1)]
    atj_hi[:, :-1] = adj_tj[:, np.arange(0, T - 1), np.arange(1, T)]
    lohi = np.zeros((2, 2, 128, FD), dtype=np.float32)
    for h in range(2):
        lohi[0, h] = np.tile(atj_lo[:, h * 128:(h + 1) * 128].T, (1, NB))
        lohi[1, h] = np.tile(atj_hi[:, h * 128:(h + 1) * 128].T, (1, NB))

    wqs = np.zeros((2, 2, 2, 128, 128), dtype=np.float32)
    for w, M in enumerate((fc_w, W2)):
        for kh in range(2):
            for F in range(2):
                wqs[w, kh, F] = M[128 * F:128 * (F + 1),
                                  128 * kh:128 * (kh + 1)].T
    arep = np.tile(alpha_v[None, :], (128, NB))
    brep = np.tile(beta_v[None, :], (128, NB))
    fcb = np.stack([fc_b[0:128, None], fc_b[128:256, None]]).astype(np.float32)

    lohi_bf = lohi.astype(BF)
    wqs_bf = wqs.astype(BF)
    arep_bf = arep.astype(BF)
    brep_bf = brep.astype(BF)

    in_maps = []
    for cidx in range(N_CORES):
        sl_ = slice(cidx * BL, (cidx + 1) * BL)
        xc = x[sl_]                                           # [BL, V, T]
        # natural packed: [NG, V, NB*T]
        xsn = np.ascontiguousarray(
            xc.reshape(NG, NB, V, T).transpose(0, 2, 1, 3)
        ).reshape(NG, V, NB * T).astype(BF)
        # transposed packed: [NG, 2, 128, NB*V]
        xst = np.ascontiguousarray(
            xc.reshape(NG, NB, V, 2, 128).transpose(0, 3, 4, 1, 2)
        ).reshape(NG, 2, 128, FD).astype(BF)
        altc = np.ascontiguousarray(
            altT[sl_].reshape(NG, NB, V, V).transpose(0, 2, 1, 3)
        ).reshape(NG, V, FD).astype(BF)
        g1c, g3c = g1[sl_], g3[sl_]
        gpat_c = np.zeros((NG, 2, FD), dtype=np.float32)
        gpat_c[:, 0, :] = np.repeat(g1c.reshape(NG, NB), V, axis=1)
        gpat_c[:, 1, :] = np.repeat(g3c.reshape(NG, NB), V, axis=1)
        in_maps.append(dict(
            xsn=xsn, xst=xst, alt=altc, gpat=gpat_c.astype(BF),
            lohi=lohi_bf, wqs=wqs_bf, arep=arep_bf, brep=brep_bf, fcb=fcb,
        ))

    nc = _build_nc(trivial_affine)
    res = run_bass_kernel_spmd(nc, in_maps, core_ids=list(range(N_CORES)),
                               **_RUN_KW)
    _LAST_RES.clear()
    _LAST_RES["res"] = res
    out = np.empty((B, V, T), dtype=np.float32)
    for cidx in range(N_CORES):
        yt = np.asarray(res.results[cidx]["yst"])             # [NG,2,128,FD] bf16
        yt = yt.reshape(NG, 2, 128, NB, V).transpose(0, 3, 4, 1, 2)
        out[cidx * BL:(cidx + 1) * BL] = yt.reshape(BL, V, T).astype(np.float32)
    return out


_RUN_KW = {}
_LAST_RES = {}
